# revision 8
# baseline (speedup 1.0000x reference)
"""Trainium2 Bass kernel for nn_Decoder_TNTM (topic-model decoder).

out[b,v] = logsumexp_k( log_beta[k,v] + log_softmax(theta_hat)[b,k] )

Math (validated against the jax reference to ~2e-7 rel):
  With Woodbury, Sigma_k^{-1} = Dinv - U_k U_k^T  (U = Dinv W Lc^{-T}),
  log_beta[k,v] = A_k + e_v.b_k + (e_v^2).c_k + 0.5||U_k^T e_v||^2
  where all K-sized coefficients (A, b, c, U, logdet, theta) are computed
  on the host in float64 (O(K d r^2) ~ 0.3% of total FLOPs).

Device work per core (V sharded 8 ways, 6272 rows/core in 49 tiles of 128):
  PE : Z = E @ Ucat  (contraction d=256, output 128v x 4096kr in PSUM),
       G = E@b + E^2@c + A (128v x 64k), transposes, final theta matmul
  ACT: Square(Z) PSUM->SBUF, exp(log_beta - m), ln(S)
  DVE: segmented reduce of z^2 over r, max_k, small fixups
"""

import numpy as np
from contextlib import ExitStack

import bass_rust
import concourse.bass as bass
import concourse.mybir as mybir
import concourse.tile as tile
from concourse.bass_utils import run_bass_kernel_spmd
from concourse.masks import make_identity
from concourse.vector_clock import ScopedClock, VectorClock

F32 = mybir.dt.float32
AF = mybir.ActivationFunctionType
ALU = mybir.AluOpType

N_CORES = 8
B, K, V, D, R = 64, 64, 50000, 256, 64
KR = K * R                       # 4096
V_PER_CORE = V // N_CORES        # 6250
N_VT = (V_PER_CORE + 127) // 128  # 49
V_PAD = N_VT * 128               # 6272
LOG_2PI = float(np.log(2.0 * np.pi))


class _SplitDrainTileContext(tile.TileContext):
    """This container's walrus rejects >1 sem wait per CTRL-class
    instruction; split the tail drain's waits across sync NOPs."""

    def _drain_and_barrier(self, tick_clock, wait_clock):
        gc = tick_clock.global_clock
        nproc = len(gc)
        for p in (i for i in range(nproc) if gc[i] > 0):
            vec = [0] * nproc
            vec[p] = gc[p]
            nop_inst = self.nc.sync.nop(nofuse=True)
            wait_clock.add_sem_waits(
                nop_inst.ins, ScopedClock({None: VectorClock(vec)})
            )
        self.nc.sync.drain()

        self.nc.all_engine_barrier()
        assert self.sems is not None
        popped = self.nc._tile_sem_poison_stack.pop()
        assert popped is self._sem_poison
        self.nc.clear_and_free_semaphores(list(self.sems.allocated().values()))
        self.nc.all_engine_barrier()


def emit(nc, tc, aps, n_vt=N_VT):
    E, U, Bc, Cc, Arow, thetaT, outT = (
        aps["E"], aps["U"], aps["Bc"], aps["Cc"],
        aps["Arow"], aps["thetaT"], aps["outT"],
    )
    with ExitStack() as ctx:
        cst = ctx.enter_context(tc.tile_pool(name="cst", bufs=1))

        # resident constants
        u0 = cst.tile([128, KR], F32)
        nc.sync.dma_start(u0[:], U[0:128, :])
        u1 = cst.tile([128, KR], F32)
        nc.sync.dma_start(u1[:], U[128:256, :])
        bc0 = cst.tile([128, K], F32)
        nc.sync.dma_start(bc0[:], Bc[0:128, :])
        bc1 = cst.tile([128, K], F32)
        nc.sync.dma_start(bc1[:], Bc[128:256, :])
        cc0 = cst.tile([128, K], F32)
        nc.sync.dma_start(cc0[:], Cc[0:128, :])
        cc1 = cst.tile([128, K], F32)
        nc.sync.dma_start(cc1[:], Cc[128:256, :])
        arow = cst.tile([1, K], F32)
        nc.sync.dma_start(arow[:], Arow[:])
        tht = cst.tile([K, B], F32)
        nc.sync.dma_start(tht[:], thetaT[:])
        ident = cst.tile([128, 128], F32)
        make_identity(nc, ident[:])
        ones1 = cst.tile([1, 128], F32)
        nc.gpsimd.memset(ones1[:], 1.0)

        ep = ctx.enter_context(tc.tile_pool(name="ep", bufs=3))
        etp = ctx.enter_context(tc.tile_pool(name="etp", bufs=4))
        z2p = ctx.enter_context(tc.tile_pool(name="z2p", bufs=2))
        smp = ctx.enter_context(tc.tile_pool(name="smp", bufs=3))
        outp = ctx.enter_context(tc.tile_pool(name="outp", bufs=3))

        zps = ctx.enter_context(tc.tile_pool(name="zps", bufs=3, space="PSUM"))
        gps = ctx.enter_context(tc.tile_pool(name="gps", bufs=2, space="PSUM"))
        tps = ctx.enter_context(tc.tile_pool(name="tps", bufs=2, space="PSUM"))
        sps = ctx.enter_context(tc.tile_pool(name="sps", bufs=1, space="PSUM"))

        def stage1(vt):
            """DMA + transposes + G + Z + squares + reduce + logb + max.
            Returns tiles needed by stage2."""
            et = ep.tile([128, D], F32, tag="et", name=f"et_{vt}")
            nc.sync.dma_start(et[:], E[vt * 128:(vt + 1) * 128, :])

            ET = []
            E2T = []
            for dh in range(2):
                tp = tps.tile([128, 128], F32, tag="tp", name=f"tp_{vt}_{dh}")
                nc.tensor.transpose(tp[:], et[:, dh * 128:(dh + 1) * 128], ident[:])
                etr = etp.tile([128, 128], F32, tag="etr", name=f"etr_{vt}_{dh}")
                nc.vector.tensor_copy(etr[:], tp[:])
                e2tr = etp.tile([128, 128], F32, tag="e2tr", name=f"e2tr_{vt}_{dh}")
                nc.scalar.activation(e2tr[:], tp[:], AF.Square)
                ET.append(etr)
                E2T.append(e2tr)

            # G[v,k] = A_k + E.b + E^2.c   (PSUM, 128v x 64k)
            g = gps.tile([128, K], F32, tag="g", name=f"g_{vt}")
            nc.tensor.matmul(g[:], ET[0][:], bc0[:], start=True, stop=False)
            nc.tensor.matmul(g[:], ET[1][:], bc1[:], start=False, stop=False)
            nc.tensor.matmul(g[:], E2T[0][:], cc0[:], start=False, stop=False)
            nc.tensor.matmul(g[:], E2T[1][:], cc1[:], start=False, stop=False)
            nc.tensor.matmul(g[:], ones1[:], arow[:], start=False, stop=True)

            # Z = E @ Ucat ; z2 = Z^2 ; partial reduce per half
            z2 = z2p.tile([128, KR], F32, tag="z2", name=f"z2_{vt}")
            s2 = smp.tile([128, K], F32, tag="s2", name=f"s2_{vt}")
            for jh in range(2):
                zp4 = [zps.tile([128, 512], F32, tag="zp", name=f"zp_{vt}_{jh}_{i}")
                       for i in range(4)]
                for dh in range(2):
                    usb = (u0, u1)[dh]
                    for jj in range(4):
                        j = jh * 4 + jj
                        nc.tensor.matmul(
                            zp4[jj][:], ET[dh][:], usb[:, j * 512:(j + 1) * 512],
                            start=(dh == 0), stop=(dh == 1),
                        )
                for jj in range(4):
                    j = jh * 4 + jj
                    nc.scalar.activation(
                        z2[:, j * 512:(j + 1) * 512], zp4[jj][:], AF.Square
                    )
                nc.vector.tensor_reduce(
                    s2[:, jh * 32:(jh + 1) * 32],
                    z2[:, jh * 2048:(jh + 1) * 2048].rearrange(
                        "p (k r) -> p k r", r=R),
                    axis=mybir.AxisListType.X, op=ALU.add,
                )

            logb = smp.tile([128, K], F32, tag="logb", name=f"logb_{vt}")
            nc.vector.tensor_tensor(logb[:], s2[:], g[:], op=ALU.add)
            mneg = smp.tile([128, 1], F32, tag="mneg", name=f"mneg_{vt}")
            nc.vector.tensor_reduce(
                mneg[:], logb[:], axis=mybir.AxisListType.X, op=ALU.max,
                negate=True,
            )
            return logb, mneg

        def stage2(vt, logb, mneg):
            """exp + transpose + theta matmul + ln + output DMA."""
            eb = smp.tile([128, K], F32, tag="eb", name=f"eb_{vt}")
            nc.scalar.activation(eb[:], logb[:], AF.Exp, bias=mneg[:], scale=1.0)

            tp2 = tps.tile([128, 128], F32, tag="tp", name=f"tp2_{vt}")
            nc.tensor.transpose(tp2[:K, :], eb[:], ident[:])
            ebt = smp.tile([K, 128], F32, tag="ebt", name=f"ebt_{vt}")
            nc.scalar.activation(ebt[:], tp2[:K, :], AF.Copy)
            sp = sps.tile([128, B], F32, tag="sp", name=f"sp_{vt}")
            nc.tensor.matmul(sp[:], ebt[:], tht[:], start=True, stop=True)

            outl = outp.tile([128, B], F32, tag="outl", name=f"outl_{vt}")
            nc.scalar.activation(outl[:], sp[:], AF.Ln)
            outr = outp.tile([128, B], F32, tag="outr", name=f"outr_{vt}")
            nc.vector.tensor_scalar(
                outr[:], outl[:], mneg[:], None, op0=ALU.subtract
            )
            nc.sync.dma_start(outT[vt * 128:(vt + 1) * 128, :], outr[:])

        pend = None
        for vt in range(n_vt):
            cur = stage1(vt)
            if pend is not None:
                stage2(vt - 1, *pend)
            pend = cur
        stage2(n_vt - 1, *pend)


def _split_multi_waits(nc, max_waits=1):
    """This container's walrus rejects instructions carrying more than one
    sem wait. Hoist excess waits onto same-engine NOPs inserted just before
    the offending instruction (same program point, so semantics unchanged)."""
    ctr = 0
    nsplit = 0
    for fn in nc.m.functions:
        for bb in fn.blocks:
            il = bb.instructions
            out = []
            changed = False
            for inst in il:
                si = inst.sync_info
                waits = list(si.on_wait) if si is not None and si.on_wait else []
                if len(waits) > max_waits:
                    nsplit += 1
                    extra = waits[max_waits:]
                    for c0 in range(0, len(extra), max_waits):
                        nop = mybir.InstNoOp(
                            name=f"waitsplit_{ctr}", ins=[], outs=[])
                        ctr += 1
                        nop.engine = inst.engine
                        nop.sync_info = bass_rust.SyncInfo(
                            on_wait=extra[c0:c0 + max_waits], on_update=[])
                        out.append(nop)
                    si.on_wait = waits[:max_waits]
                    changed = True
                out.append(inst)
            if changed:
                il[:] = out
    return nsplit


def build_program(n_vt=N_VT, split_waits=True):
    nc = bass.Bass("TRN2", target_bir_lowering=False, debug=False)
    aps = {
        "E": nc.dram_tensor("E", [V_PAD, D], F32, kind="ExternalInput").ap(),
        "U": nc.dram_tensor("U", [D, KR], F32, kind="ExternalInput").ap(),
        "Bc": nc.dram_tensor("Bc", [D, K], F32, kind="ExternalInput").ap(),
        "Cc": nc.dram_tensor("Cc", [D, K], F32, kind="ExternalInput").ap(),
        "Arow": nc.dram_tensor("Arow", [1, K], F32, kind="ExternalInput").ap(),
        "thetaT": nc.dram_tensor("thetaT", [K, B], F32, kind="ExternalInput").ap(),
        "outT": nc.dram_tensor("outT", [V_PAD, B], F32, kind="ExternalOutput").ap(),
    }
    with _SplitDrainTileContext(nc) as tc:
        emit(nc, tc, aps, n_vt=n_vt)
    if split_waits:
        _split_multi_waits(nc)
    return nc


def host_precompute(theta_hat, mus, L_lower, log_diag):
    """All K-sized coefficients, in float64. Returns fp32 device arrays."""
    th = theta_hat.astype(np.float64)
    mus = mus.astype(np.float64)
    L = L_lower.astype(np.float64)
    ld = log_diag.astype(np.float64)
    Kk, d, r = L.shape

    Dinv = np.exp(-ld)                                   # (K,d)
    Wd = L * Dinv[:, :, None]                            # (K,d,r)
    C = np.eye(r)[None] + np.einsum("kdr,kds->krs", L, Wd)
    Lc = np.linalg.cholesky(C)
    logdet = ld.sum(-1) + 2.0 * np.log(
        np.diagonal(Lc, axis1=-2, axis2=-1)).sum(-1)
    Lc_inv = np.linalg.inv(Lc)                           # (K,r,r)
    U = np.einsum("kdr,ksr->kds", Wd, Lc_inv)            # Wd @ Lc^{-T}
    alpha = np.einsum("kdr,kd->kr", U, mus)
    bcoef = Dinv * mus - np.einsum("kdr,kr->kd", U, alpha)
    ccoef = -0.5 * Dinv
    A = (-0.5 * (d * LOG_2PI + logdet
                 + np.einsum("kd,kd->k", Dinv * mus, mus))
         + 0.5 * (alpha ** 2).sum(-1))
    Us = U / np.sqrt(2.0)

    theta = np.exp(th - th.max(-1, keepdims=True))
    theta /= theta.sum(-1, keepdims=True)

    return {
        "U": np.ascontiguousarray(
            Us.transpose(1, 0, 2).reshape(d, Kk * r).astype(np.float32)),
        "Bc": np.ascontiguousarray(bcoef.T.astype(np.float32)),
        "Cc": np.ascontiguousarray(ccoef.T.astype(np.float32)),
        "Arow": np.ascontiguousarray(A.astype(np.float32)[None, :]),
        "thetaT": np.ascontiguousarray(theta.T.astype(np.float32)),
    }


def make_in_maps(embeddings, pre):
    in_maps = []
    for c in range(N_CORES):
        esl = np.zeros((V_PAD, D), np.float32)
        esl[:V_PER_CORE] = embeddings[c * V_PER_CORE:(c + 1) * V_PER_CORE]
        in_maps.append({"E": esl, **pre})
    return in_maps


_NC_CACHE = None


def kernel(theta_hat, embeddings, mus, L_lower, log_diag):
    global _NC_CACHE
    pre = host_precompute(theta_hat, mus, L_lower, log_diag)
    if _NC_CACHE is None:
        _NC_CACHE = build_program()
    nc = _NC_CACHE
    in_maps = make_in_maps(np.asarray(embeddings, dtype=np.float32), pre)
    res = run_bass_kernel_spmd(nc, in_maps, list(range(N_CORES)))
    out = np.empty((B, V), np.float32)
    for c in range(N_CORES):
        out[:, c * V_PER_CORE:(c + 1) * V_PER_CORE] = \
            res.results[c]["outT"][:V_PER_CORE].T
    return out


# revision 11
# speedup vs baseline: 2.0859x; 2.0859x over previous
"""Trainium2 Bass kernel for nn_Decoder_TNTM (topic-model decoder).

out[b,v] = logsumexp_k( log_beta[k,v] + log_softmax(theta_hat)[b,k] )

Math (validated against the jax reference to ~2e-7 rel):
  With Woodbury, Sigma_k^{-1} = Dinv - U_k U_k^T  (U = Dinv W Lc^{-T}),
  log_beta[k,v] = A_k + e_v.b_k + (e_v^2).c_k + 0.5||U_k^T e_v||^2
  where all K-sized coefficients (A, b, c, U, logdet, theta) are computed
  on the host in float64 (O(K d r^2) ~ 0.3% of total FLOPs).

Device work per core (V sharded 8 ways, 6272 rows/core in 49 tiles of 128):
  PE : Z = E @ Ucat  (contraction d=256, output 128v x 4096kr in PSUM),
       G = E@b + E^2@c + A (128v x 64k), transposes, final theta matmul
  ACT: Square(Z) PSUM->SBUF, exp(log_beta - m), ln(S)
  DVE: segmented reduce of z^2 over r, max_k, small fixups
"""

import numpy as np
from contextlib import ExitStack

import bass_rust
import concourse.bass as bass
import concourse.mybir as mybir
import concourse.tile as tile
from concourse.bass_utils import run_bass_kernel_spmd
from concourse.masks import make_identity
from concourse.vector_clock import ScopedClock, VectorClock

F32 = mybir.dt.float32
F32R = mybir.dt.float32r
AF = mybir.ActivationFunctionType
ALU = mybir.AluOpType

N_CORES = 8
B, K, V, D, R = 64, 64, 50000, 256, 64
KR = K * R                       # 4096
V_PER_CORE = V // N_CORES        # 6250
N_VT = (V_PER_CORE + 127) // 128  # 49
V_PAD = N_VT * 128               # 6272
LOG_2PI = float(np.log(2.0 * np.pi))


class _SplitDrainTileContext(tile.TileContext):
    """This container's walrus rejects >1 sem wait per CTRL-class
    instruction; split the tail drain's waits across sync NOPs."""

    def _drain_and_barrier(self, tick_clock, wait_clock):
        gc = tick_clock.global_clock
        nproc = len(gc)
        for p in (i for i in range(nproc) if gc[i] > 0):
            vec = [0] * nproc
            vec[p] = gc[p]
            nop_inst = self.nc.sync.nop(nofuse=True)
            wait_clock.add_sem_waits(
                nop_inst.ins, ScopedClock({None: VectorClock(vec)})
            )
        self.nc.sync.drain()

        self.nc.all_engine_barrier()
        assert self.sems is not None
        popped = self.nc._tile_sem_poison_stack.pop()
        assert popped is self._sem_poison
        self.nc.clear_and_free_semaphores(list(self.sems.allocated().values()))
        self.nc.all_engine_barrier()


def emit(nc, tc, aps, n_vt=N_VT):
    E, U, Bc, Cc, Arow, thetaT, outT = (
        aps["E"], aps["U"], aps["Bc"], aps["Cc"],
        aps["Arow"], aps["thetaT"], aps["outT"],
    )
    with ExitStack() as ctx:
        cst = ctx.enter_context(tc.tile_pool(name="cst", bufs=1))

        # resident constants (f32 staging -> f32r rounded residents for PE)
        u0f = cst.tile([128, KR], F32)
        nc.sync.dma_start(u0f[:], U[0:128, :])
        u1f = cst.tile([128, KR], F32)
        nc.sync.dma_start(u1f[:], U[128:256, :])
        u0 = cst.tile([128, KR], F32R)
        nc.vector.tensor_copy(u0[:], u0f[:])
        u1 = cst.tile([128, KR], F32R)
        nc.vector.tensor_copy(u1[:], u1f[:])
        bcf = cst.tile([128, 2 * K], F32)
        nc.sync.dma_start(bcf[:, 0:K], Bc[0:128, :])
        nc.sync.dma_start(bcf[:, K:2 * K], Bc[128:256, :])
        ccf = cst.tile([128, 2 * K], F32)
        nc.sync.dma_start(ccf[:, 0:K], Cc[0:128, :])
        nc.sync.dma_start(ccf[:, K:2 * K], Cc[128:256, :])
        bcr = cst.tile([128, 2 * K], F32R)
        nc.vector.tensor_copy(bcr[:], bcf[:])
        ccr = cst.tile([128, 2 * K], F32R)
        nc.vector.tensor_copy(ccr[:], ccf[:])
        bc0, bc1 = bcr[:, 0:K], bcr[:, K:2 * K]
        cc0, cc1 = ccr[:, 0:K], ccr[:, K:2 * K]
        arowf = cst.tile([1, K], F32)
        nc.sync.dma_start(arowf[:], Arow[:])
        arow = cst.tile([1, K], F32R)
        nc.vector.tensor_copy(arow[:], arowf[:])
        tht = cst.tile([K, B], F32)
        nc.sync.dma_start(tht[:], thetaT[:])
        ident = cst.tile([128, 128], F32)
        make_identity(nc, ident[:])
        ones1f = cst.tile([1, 128], F32)
        nc.gpsimd.memset(ones1f[:], 1.0)
        ones1 = cst.tile([1, 128], F32R)
        nc.vector.tensor_copy(ones1[:], ones1f[:])

        ep = ctx.enter_context(tc.tile_pool(name="ep", bufs=3))
        etp = ctx.enter_context(tc.tile_pool(name="etp", bufs=4))
        z2p = ctx.enter_context(tc.tile_pool(name="z2p", bufs=2))
        smp = ctx.enter_context(tc.tile_pool(name="smp", bufs=3))
        outp = ctx.enter_context(tc.tile_pool(name="outp", bufs=3))

        zps = ctx.enter_context(tc.tile_pool(name="zps", bufs=2, space="PSUM"))
        gps = ctx.enter_context(tc.tile_pool(name="gps", bufs=2, space="PSUM"))
        tps = ctx.enter_context(tc.tile_pool(name="tps", bufs=2, space="PSUM"))

        def stage1(vt):
            """DMA + transposes + G + Z + squares + reduce + logb + max.
            Returns tiles needed by stage2."""
            et = ep.tile([128, D], F32, tag="et", name=f"et_{vt}")
            nc.sync.dma_start(et[:], E[vt * 128:(vt + 1) * 128, :])

            ET = []
            E2T = []
            for dh in range(2):
                tp = tps.tile([128, 128], F32, tag="tp", name=f"tp_{vt}_{dh}")
                nc.tensor.transpose(tp[:], et[:, dh * 128:(dh + 1) * 128], ident[:])
                etr = etp.tile([128, 128], F32R, tag="etr", name=f"etr_{vt}_{dh}")
                nc.scalar.activation(etr[:], tp[:], AF.Copy)
                e2tr = etp.tile([128, 128], F32R, tag="e2tr", name=f"e2tr_{vt}_{dh}")
                nc.scalar.activation(e2tr[:], tp[:], AF.Square)
                ET.append(etr)
                E2T.append(e2tr)

            # G[v,k] = A_k + E.b + E^2.c   (PSUM, 128v x 64k)
            g = gps.tile([128, K], F32, tag="gs", name=f"g_{vt}")
            nc.tensor.matmul(g[:], ET[0][:], bc0, start=True, stop=False)
            nc.tensor.matmul(g[:], ET[1][:], bc1, start=False, stop=False)
            nc.tensor.matmul(g[:], E2T[0][:], cc0, start=False, stop=False)
            nc.tensor.matmul(g[:], E2T[1][:], cc1, start=False, stop=False)
            nc.tensor.matmul(g[:], ones1[:], arow[:], start=False, stop=True)

            # Z = E @ Ucat ; z2 = Z^2 ; partial reduce per half
            z2 = z2p.tile([128, KR], F32, tag="z2", name=f"z2_{vt}")
            s2 = smp.tile([128, K], F32, tag="s2", name=f"s2_{vt}")
            for jh in range(2):
                zp2 = [zps.tile([128, 1024], F32, tag="zp", name=f"zp_{vt}_{jh}_{i}")
                       for i in range(2)]
                for dh in range(2):
                    usb = (u0, u1)[dh]
                    for jj in range(4):
                        j = jh * 4 + jj
                        nc.tensor.matmul(
                            zp2[jj // 2][:, (jj % 2) * 512:(jj % 2) * 512 + 512],
                            ET[dh][:], usb[:, j * 512:(j + 1) * 512],
                            start=(dh == 0), stop=(dh == 1),
                        )
                for jj in range(2):
                    j2 = jh * 2 + jj
                    nc.scalar.activation(
                        z2[:, j2 * 1024:(j2 + 1) * 1024], zp2[jj][:], AF.Square
                    )
                nc.vector.tensor_reduce(
                    s2[:, jh * 32:(jh + 1) * 32],
                    z2[:, jh * 2048:(jh + 1) * 2048].rearrange(
                        "p (k r) -> p k r", r=R),
                    axis=mybir.AxisListType.X, op=ALU.add,
                )

            logb = smp.tile([128, K], F32, tag="logb", name=f"logb_{vt}")
            nc.vector.tensor_tensor(logb[:], s2[:], g[:], op=ALU.add)
            mneg = smp.tile([128, 1], F32, tag="mneg", name=f"mneg_{vt}")
            nc.vector.tensor_reduce(
                mneg[:], logb[:], axis=mybir.AxisListType.X, op=ALU.max,
                negate=True,
            )
            return logb, mneg

        def stage2(vt, logb, mneg):
            """exp + transpose + theta matmul + ln + output DMA."""
            eb = smp.tile([128, K], F32, tag="eb", name=f"eb_{vt}")
            nc.scalar.activation(eb[:], logb[:], AF.Exp, bias=mneg[:], scale=1.0)

            tp2 = tps.tile([128, 128], F32, tag="tp", name=f"tp2_{vt}")
            nc.tensor.transpose(tp2[:K, :], eb[:], ident[:])
            ebt = smp.tile([K, 128], F32, tag="ebt", name=f"ebt_{vt}")
            nc.scalar.activation(ebt[:], tp2[:K, :], AF.Copy)
            sp = gps.tile([128, B], F32, tag="gs", name=f"sp_{vt}")
            nc.tensor.matmul(sp[:], ebt[:], tht[:], start=True, stop=True)

            outl = outp.tile([128, B], F32, tag="outl", name=f"outl_{vt}")
            nc.scalar.activation(outl[:], sp[:], AF.Ln)
            outr = outp.tile([128, B], F32, tag="outr", name=f"outr_{vt}")
            nc.vector.tensor_scalar(
                outr[:], outl[:], mneg[:], None, op0=ALU.subtract
            )
            nc.sync.dma_start(outT[vt * 128:(vt + 1) * 128, :], outr[:])

        pend = None
        for vt in range(n_vt):
            cur = stage1(vt)
            if pend is not None:
                stage2(vt - 1, *pend)
            pend = cur
        stage2(n_vt - 1, *pend)


def _split_multi_waits(nc, max_waits=1):
    """This container's walrus rejects instructions carrying more than one
    sem wait. Hoist excess waits onto same-engine NOPs inserted just before
    the offending instruction (same program point, so semantics unchanged)."""
    ctr = 0
    nsplit = 0
    for fn in nc.m.functions:
        for bb in fn.blocks:
            il = bb.instructions
            out = []
            changed = False
            for inst in il:
                si = inst.sync_info
                waits = list(si.on_wait) if si is not None and si.on_wait else []
                if len(waits) > max_waits:
                    nsplit += 1
                    extra = waits[max_waits:]
                    for c0 in range(0, len(extra), max_waits):
                        nop = mybir.InstNoOp(
                            name=f"waitsplit_{ctr}", ins=[], outs=[])
                        ctr += 1
                        nop.engine = inst.engine
                        nop.sync_info = bass_rust.SyncInfo(
                            on_wait=extra[c0:c0 + max_waits], on_update=[])
                        out.append(nop)
                    si.on_wait = waits[:max_waits]
                    changed = True
                out.append(inst)
            if changed:
                il[:] = out
    return nsplit


def build_program(n_vt=N_VT, split_waits=True):
    nc = bass.Bass("TRN2", target_bir_lowering=False, debug=False)
    aps = {
        "E": nc.dram_tensor("E", [V_PAD, D], F32, kind="ExternalInput").ap(),
        "U": nc.dram_tensor("U", [D, KR], F32, kind="ExternalInput").ap(),
        "Bc": nc.dram_tensor("Bc", [D, K], F32, kind="ExternalInput").ap(),
        "Cc": nc.dram_tensor("Cc", [D, K], F32, kind="ExternalInput").ap(),
        "Arow": nc.dram_tensor("Arow", [1, K], F32, kind="ExternalInput").ap(),
        "thetaT": nc.dram_tensor("thetaT", [K, B], F32, kind="ExternalInput").ap(),
        "outT": nc.dram_tensor("outT", [V_PAD, B], F32, kind="ExternalOutput").ap(),
    }
    with _SplitDrainTileContext(nc) as tc:
        emit(nc, tc, aps, n_vt=n_vt)
    if split_waits:
        _split_multi_waits(nc)
    return nc


def host_precompute(theta_hat, mus, L_lower, log_diag):
    """All K-sized coefficients, in float64. Returns fp32 device arrays."""
    th = theta_hat.astype(np.float64)
    mus = mus.astype(np.float64)
    L = L_lower.astype(np.float64)
    ld = log_diag.astype(np.float64)
    Kk, d, r = L.shape

    Dinv = np.exp(-ld)                                   # (K,d)
    Wd = L * Dinv[:, :, None]                            # (K,d,r)
    C = np.eye(r)[None] + np.einsum("kdr,kds->krs", L, Wd)
    Lc = np.linalg.cholesky(C)
    logdet = ld.sum(-1) + 2.0 * np.log(
        np.diagonal(Lc, axis1=-2, axis2=-1)).sum(-1)
    Lc_inv = np.linalg.inv(Lc)                           # (K,r,r)
    U = np.einsum("kdr,ksr->kds", Wd, Lc_inv)            # Wd @ Lc^{-T}
    alpha = np.einsum("kdr,kd->kr", U, mus)
    bcoef = Dinv * mus - np.einsum("kdr,kr->kd", U, alpha)
    ccoef = -0.5 * Dinv
    A = (-0.5 * (d * LOG_2PI + logdet
                 + np.einsum("kd,kd->k", Dinv * mus, mus))
         + 0.5 * (alpha ** 2).sum(-1))
    Us = U / np.sqrt(2.0)

    theta = np.exp(th - th.max(-1, keepdims=True))
    theta /= theta.sum(-1, keepdims=True)

    return {
        "U": np.ascontiguousarray(
            Us.transpose(1, 0, 2).reshape(d, Kk * r).astype(np.float32)),
        "Bc": np.ascontiguousarray(bcoef.T.astype(np.float32)),
        "Cc": np.ascontiguousarray(ccoef.T.astype(np.float32)),
        "Arow": np.ascontiguousarray(A.astype(np.float32)[None, :]),
        "thetaT": np.ascontiguousarray(theta.T.astype(np.float32)),
    }


def make_in_maps(embeddings, pre):
    in_maps = []
    for c in range(N_CORES):
        esl = np.zeros((V_PAD, D), np.float32)
        esl[:V_PER_CORE] = embeddings[c * V_PER_CORE:(c + 1) * V_PER_CORE]
        in_maps.append({"E": esl, **pre})
    return in_maps


_NC_CACHE = None


def kernel(theta_hat, embeddings, mus, L_lower, log_diag):
    global _NC_CACHE
    pre = host_precompute(theta_hat, mus, L_lower, log_diag)
    if _NC_CACHE is None:
        _NC_CACHE = build_program()
    nc = _NC_CACHE
    in_maps = make_in_maps(np.asarray(embeddings, dtype=np.float32), pre)
    res = run_bass_kernel_spmd(nc, in_maps, list(range(N_CORES)))
    out = np.empty((B, V), np.float32)
    for c in range(N_CORES):
        out[:, c * V_PER_CORE:(c + 1) * V_PER_CORE] = \
            res.results[c]["outT"][:V_PER_CORE].T
    return out


# revision 12
# speedup vs baseline: 2.7516x; 1.3192x over previous
"""Trainium2 Bass kernel for nn_Decoder_TNTM (topic-model decoder).

out[b,v] = logsumexp_k( log_beta[k,v] + log_softmax(theta_hat)[b,k] )

Math (validated against the jax reference):
  With Woodbury, Sigma_k^{-1} = Dinv - U_k U_k^T  (U = Dinv W Lc^{-T}),
  log_beta[k,v] = A_k + e_v.b_k + (e_v^2).c_k + 0.5||U_k^T e_v||^2.
  K-sized coefficients (A, b, c, U, theta) are computed on the host in
  float64 (~0.3% of FLOPs). A_k is folded into theta:
  theta'[k,b] = softmax(theta_hat)[b,k] * exp(A_k - maxA), compensated by
  +maxA on the output, which is safe because spread(A) + |log theta| stays
  far above the fp32 underflow exponent.

Device work per core (V sharded 8 ways, 6272 rows/core in 49 tiles of 128):
  PE : Z = E @ Ucat as fp32r (RNE-11 mantissa, full-rate; fp32 is 1/4 rate),
       G' = E@b + E^2@c, transpose of exp-row, final theta matmul
  ACT: Square(Z) PSUM->SBUF, exp(log_beta' - m), ln(S), small copies
  DVE: segmented reduce of z^2 over r (the 1x-rate floor), max_k, fixups
  E^T and (E^T)^2 are uploaded pre-transposed from the host as fp32r
  (the PE rounds fp32r operands on ingest; measured RNE-11).
"""

import numpy as np
from contextlib import ExitStack

import bass_rust
import concourse.bass as bass
import concourse.mybir as mybir
import concourse.tile as tile
from concourse.bass_utils import run_bass_kernel_spmd
from concourse.masks import make_identity
from concourse.vector_clock import ScopedClock, VectorClock

F32 = mybir.dt.float32
F32R = mybir.dt.float32r
AF = mybir.ActivationFunctionType
ALU = mybir.AluOpType

N_CORES = 8
B, K, V, D, R = 64, 64, 50000, 256, 64
KR = K * R                       # 4096
V_PER_CORE = V // N_CORES        # 6250
N_VT = (V_PER_CORE + 127) // 128  # 49
V_PAD = N_VT * 128               # 6272
LOG_2PI = float(np.log(2.0 * np.pi))


class _SplitDrainTileContext(tile.TileContext):
    """This container's walrus rejects >1 sem wait per CTRL-class
    instruction; split the tail drain's waits across sync NOPs."""

    def _drain_and_barrier(self, tick_clock, wait_clock):
        gc = tick_clock.global_clock
        nproc = len(gc)
        for p in (i for i in range(nproc) if gc[i] > 0):
            vec = [0] * nproc
            vec[p] = gc[p]
            nop_inst = self.nc.sync.nop(nofuse=True)
            wait_clock.add_sem_waits(
                nop_inst.ins, ScopedClock({None: VectorClock(vec)})
            )
        self.nc.sync.drain()

        self.nc.all_engine_barrier()
        assert self.sems is not None
        popped = self.nc._tile_sem_poison_stack.pop()
        assert popped is self._sem_poison
        self.nc.clear_and_free_semaphores(list(self.sems.allocated().values()))
        self.nc.all_engine_barrier()


def _split_multi_waits(nc, max_waits=1):
    """Walrus here rejects instructions carrying more than one sem wait.
    Hoist excess waits onto same-engine NOPs inserted just before the
    offending instruction (same program point, so semantics unchanged)."""
    ctr = 0
    nsplit = 0
    for fn in nc.m.functions:
        for bb in fn.blocks:
            il = bb.instructions
            out = []
            changed = False
            for inst in il:
                si = inst.sync_info
                waits = list(si.on_wait) if si is not None and si.on_wait else []
                if len(waits) > max_waits:
                    nsplit += 1
                    extra = waits[max_waits:]
                    for c0 in range(0, len(extra), max_waits):
                        nop = mybir.InstNoOp(
                            name=f"waitsplit_{ctr}", ins=[], outs=[])
                        ctr += 1
                        nop.engine = inst.engine
                        nop.sync_info = bass_rust.SyncInfo(
                            on_wait=extra[c0:c0 + max_waits], on_update=[])
                        out.append(nop)
                    si.on_wait = waits[:max_waits]
                    changed = True
                out.append(inst)
            if changed:
                il[:] = out
    return nsplit


AMAX = 0.0  # set per-build; compensation for the A-fold into theta


def emit(nc, tc, aps, n_vt=N_VT):
    ET, E2T, U, Bc, Cc, thetaT, outT = (
        aps["ET"], aps["E2T"], aps["U"], aps["Bc"], aps["Cc"],
        aps["thetaT"], aps["outT"],
    )
    with ExitStack() as ctx:
        cst = ctx.enter_context(tc.tile_pool(name="cst", bufs=1))

        # resident constants (fp32r for PE operands; PE rounds on ingest)
        u0 = cst.tile([128, KR], F32R)
        nc.sync.dma_start(u0[:], U[0:128, :])
        u1 = cst.tile([128, KR], F32R)
        nc.sync.dma_start(u1[:], U[128:256, :])
        bcr = cst.tile([128, 2 * K], F32R)
        nc.sync.dma_start(bcr[:, 0:K], Bc[0:128, :])
        nc.sync.dma_start(bcr[:, K:2 * K], Bc[128:256, :])
        ccr = cst.tile([128, 2 * K], F32R)
        nc.sync.dma_start(ccr[:, 0:K], Cc[0:128, :])
        nc.sync.dma_start(ccr[:, K:2 * K], Cc[128:256, :])
        bc0, bc1 = bcr[:, 0:K], bcr[:, K:2 * K]
        cc0, cc1 = ccr[:, 0:K], ccr[:, K:2 * K]
        tht = cst.tile([K, B], F32)
        nc.sync.dma_start(tht[:], thetaT[:])
        ident = cst.tile([128, 128], F32)
        make_identity(nc, ident[:])

        etp = ctx.enter_context(tc.tile_pool(name="etp", bufs=3))
        z2p = ctx.enter_context(tc.tile_pool(name="z2p", bufs=2))
        smp = ctx.enter_context(tc.tile_pool(name="smp", bufs=3))
        outp = ctx.enter_context(tc.tile_pool(name="outp", bufs=3))

        zps = ctx.enter_context(tc.tile_pool(name="zps", bufs=2, space="PSUM"))
        gps = ctx.enter_context(tc.tile_pool(name="gps", bufs=3, space="PSUM"))

        def stage1(vt):
            sl = slice(vt * 128, (vt + 1) * 128)
            et0 = etp.tile([128, 128], F32R, tag="et0", name=f"et0_{vt}")
            nc.sync.dma_start(et0[:], ET[0:128, sl])
            et1 = etp.tile([128, 128], F32R, tag="et1", name=f"et1_{vt}")
            nc.sync.dma_start(et1[:], ET[128:256, sl])
            e2t0 = etp.tile([128, 128], F32R, tag="e2t0", name=f"e2t0_{vt}")
            nc.sync.dma_start(e2t0[:], E2T[0:128, sl])
            e2t1 = etp.tile([128, 128], F32R, tag="e2t1", name=f"e2t1_{vt}")
            nc.sync.dma_start(e2t1[:], E2T[128:256, sl])

            # Z = E @ Ucat ; z2 = Z^2 ; partial reduce per half
            z2 = z2p.tile([128, KR], F32, tag="z2", name=f"z2_{vt}")
            s2 = smp.tile([128, K], F32, tag="s2", name=f"s2_{vt}")
            for jh in range(2):
                zp2 = [zps.tile([128, 1024], F32, tag="zp", name=f"zp_{vt}_{jh}_{i}")
                       for i in range(2)]
                for dh in range(2):
                    usb = (u0, u1)[dh]
                    etd = (et0, et1)[dh]
                    for jj in range(4):
                        j = jh * 4 + jj
                        nc.tensor.matmul(
                            zp2[jj // 2][:, (jj % 2) * 512:(jj % 2) * 512 + 512],
                            etd[:], usb[:, j * 512:(j + 1) * 512],
                            start=(dh == 0), stop=(dh == 1),
                        )
                for jj in range(2):
                    j2 = jh * 2 + jj
                    nc.scalar.activation(
                        z2[:, j2 * 1024:(j2 + 1) * 1024], zp2[jj][:], AF.Square
                    )
                nc.vector.tensor_reduce(
                    s2[:, jh * 32:(jh + 1) * 32],
                    z2[:, jh * 2048:(jh + 1) * 2048].rearrange(
                        "p (k r) -> p k r", r=R),
                    axis=mybir.AxisListType.X, op=ALU.add,
                )

            # G'[v,k] = E.b + E^2.c   (PSUM, 128v x 64k)
            g = gps.tile([128, 128], F32, tag="gs", name=f"g_{vt}")
            nc.tensor.matmul(g[:, :K], et0[:], bc0, start=True, stop=False)
            nc.tensor.matmul(g[:, :K], et1[:], bc1, start=False, stop=False)
            nc.tensor.matmul(g[:, :K], e2t0[:], cc0, start=False, stop=False)
            nc.tensor.matmul(g[:, :K], e2t1[:], cc1, start=False, stop=True)

            logb = smp.tile([128, K], F32, tag="logb", name=f"logb_{vt}")
            nc.vector.tensor_tensor(logb[:], s2[:], g[:, :K], op=ALU.add)
            mneg = smp.tile([128, 1], F32, tag="mneg", name=f"mneg_{vt}")
            nc.vector.tensor_reduce(
                mneg[:], logb[:], axis=mybir.AxisListType.X, op=ALU.max,
                negate=True,
            )
            return logb, mneg

        def stage2(vt, logb, mneg):
            eb = smp.tile([128, K], F32, tag="eb", name=f"eb_{vt}")
            nc.scalar.activation(eb[:], logb[:], AF.Exp, bias=mneg[:], scale=1.0)

            tp2 = gps.tile([128, 128], F32, tag="gs", name=f"tp2_{vt}")
            nc.tensor.transpose(tp2[:K, :], eb[:], ident[:])
            ebt = smp.tile([K, 128], F32, tag="ebt", name=f"ebt_{vt}")
            nc.scalar.activation(ebt[:], tp2[:K, :], AF.Copy)
            sp = gps.tile([128, 128], F32, tag="gs", name=f"sp_{vt}")
            nc.tensor.matmul(sp[:, :B], ebt[:], tht[:], start=True, stop=True)

            # out = ln(S) + m + maxA
            outl = outp.tile([128, B], F32, tag="outl", name=f"outl_{vt}")
            nc.scalar.activation(outl[:], sp[:, :B], AF.Ln)
            outr = outp.tile([128, B], F32, tag="outr", name=f"outr_{vt}")
            nc.vector.tensor_scalar(
                outr[:], outl[:], mneg[:], AMAX,
                op0=ALU.subtract, op1=ALU.add,
            )
            nc.sync.dma_start(outT[vt * 128:(vt + 1) * 128, :], outr[:])

        pend = None
        for vt in range(n_vt):
            cur = stage1(vt)
            if pend is not None:
                stage2(vt - 1, *pend)
            pend = cur
        stage2(n_vt - 1, *pend)


def build_program(n_vt=N_VT, split_waits=True, amax=0.0):
    global AMAX
    AMAX = float(amax)
    nc = bass.Bass("TRN2", target_bir_lowering=False, debug=False)
    aps = {
        "ET": nc.dram_tensor("ET", [D, V_PAD], F32R, kind="ExternalInput").ap(),
        "E2T": nc.dram_tensor("E2T", [D, V_PAD], F32R, kind="ExternalInput").ap(),
        "U": nc.dram_tensor("U", [D, KR], F32R, kind="ExternalInput").ap(),
        "Bc": nc.dram_tensor("Bc", [D, K], F32R, kind="ExternalInput").ap(),
        "Cc": nc.dram_tensor("Cc", [D, K], F32R, kind="ExternalInput").ap(),
        "thetaT": nc.dram_tensor("thetaT", [K, B], F32, kind="ExternalInput").ap(),
        "outT": nc.dram_tensor("outT", [V_PAD, B], F32, kind="ExternalOutput").ap(),
    }
    with _SplitDrainTileContext(nc) as tc:
        emit(nc, tc, aps, n_vt=n_vt)
    if split_waits:
        _split_multi_waits(nc)
    return nc


def host_precompute(theta_hat, mus, L_lower, log_diag):
    """All K-sized coefficients, in float64. Returns fp32 device arrays
    plus the amax compensation scalar."""
    th = np.asarray(theta_hat).astype(np.float64)
    mus = np.asarray(mus).astype(np.float64)
    L = np.asarray(L_lower).astype(np.float64)
    ld = np.asarray(log_diag).astype(np.float64)
    Kk, d, r = L.shape

    Dinv = np.exp(-ld)                                   # (K,d)
    Wd = L * Dinv[:, :, None]                            # (K,d,r)
    C = np.eye(r)[None] + np.einsum("kdr,kds->krs", L, Wd)
    Lc = np.linalg.cholesky(C)
    logdet = ld.sum(-1) + 2.0 * np.log(
        np.diagonal(Lc, axis1=-2, axis2=-1)).sum(-1)
    Lc_inv = np.linalg.inv(Lc)                           # (K,r,r)
    U = np.einsum("kdr,ksr->kds", Wd, Lc_inv)            # Wd @ Lc^{-T}
    alpha = np.einsum("kdr,kd->kr", U, mus)
    bcoef = Dinv * mus - np.einsum("kdr,kr->kd", U, alpha)
    ccoef = -0.5 * Dinv
    A = (-0.5 * (d * LOG_2PI + logdet
                 + np.einsum("kd,kd->k", Dinv * mus, mus))
         + 0.5 * (alpha ** 2).sum(-1))
    Us = U / np.sqrt(2.0)

    theta = np.exp(th - th.max(-1, keepdims=True))
    theta /= theta.sum(-1, keepdims=True)
    amax = A.max()
    thetaA = theta.T * np.exp(A - amax)[:, None]          # (K,B)

    return {
        "U": np.ascontiguousarray(
            Us.transpose(1, 0, 2).reshape(d, Kk * r).astype(np.float32)),
        "Bc": np.ascontiguousarray(bcoef.T.astype(np.float32)),
        "Cc": np.ascontiguousarray(ccoef.T.astype(np.float32)),
        "thetaT": np.ascontiguousarray(thetaA.astype(np.float32)),
    }, float(amax)


def make_in_maps(embeddings, pre):
    emb = np.asarray(embeddings, dtype=np.float32)
    in_maps = []
    for c in range(N_CORES):
        esl = np.zeros((V_PAD, D), np.float32)
        esl[:V_PER_CORE] = emb[c * V_PER_CORE:(c + 1) * V_PER_CORE]
        et = np.ascontiguousarray(esl.T)                 # (D, V_PAD)
        in_maps.append({
            "ET": et,
            "E2T": np.ascontiguousarray(et * et),
            **pre,
        })
    return in_maps


_NC_CACHE = None
_NC_CACHE_AMAX = None


def kernel(theta_hat, embeddings, mus, L_lower, log_diag):
    global _NC_CACHE, _NC_CACHE_AMAX
    pre, amax = host_precompute(theta_hat, mus, L_lower, log_diag)
    if _NC_CACHE is None or _NC_CACHE_AMAX != amax:
        _NC_CACHE = build_program(amax=amax)
        _NC_CACHE_AMAX = amax
    nc = _NC_CACHE
    in_maps = make_in_maps(embeddings, pre)
    res = run_bass_kernel_spmd(nc, in_maps, list(range(N_CORES)))
    out = np.empty((B, V), np.float32)
    for c in range(N_CORES):
        out[:, c * V_PER_CORE:(c + 1) * V_PER_CORE] = \
            res.results[c]["outT"][:V_PER_CORE].T
    return out
